# revision 45
# baseline (speedup 1.0000x reference)
"""GaussianMLP sampling kernel for 8 trn2 NeuronCores (pure data parallel).

reference:
    h      = relu(x @ W_emb + b_emb)        x:[B,128] W_emb:[128,256]
    mean   = h @ W_mean + b_mean            W_mean:[256,128]
    logvar = h @ W_logvar + b_logvar        W_logvar:[256,128]
    z      = mean + exp(0.5*logvar) * eps
    returns (z, mean, logvar)

Design (transposed dataflow; fp8-e3m4 x, bf16 everything else):
  - Host ships x as fp8-e3m4 scaled by 2 (compensated by We/2, exact in
    bf16): worst rel-to-scale error ~1.5e-2 vs the 2e-2 gate, and input
    traffic drops 8 MiB/core.  Per 4096-row tile the combined input
    stream is [x-fp8-bytes | epsT-bf16] = ONE 1.5 MiB load (SP HWDGE
    ring); outputs are staged as one [128, 12288] tile (z|mean|logvar)
    and stored with ONE 3 MiB SWDGE DMA issued from GpSimd — the engine
    that finishes each tile last — so no compute engine's queue ever
    head-of-line blocks on a store, and stores never block loads.
  - Weights/biases are packed into two small blobs (1 bf16 + 1 f32 DMA).
  - Software pipeline with SKEW=3 and a per-step interleaved emission
    order chosen so every engine's in-order queue starts with long-ready
    work and producers run just-in-time:
      PE : [L1h0(k), L2mean(j), L1h1(k), L2logvar(j)]   (j = k-SKEW)
      ACT: [mean_out(j), relu_h0(k), std(j)]
      DVE: [z(j-2), logvar_out(j), relu_h1(k)]
      GPS: [se(j-1), store]
    se and z are deferred 1 and 2 steps past their producers so the
    std(ACT) -> se(DVE) -> z(GPS) chain never stalls a queue.
  - std reads the bf16 logvar staging copy (SBUF) instead of l_ps, so
    each L2 psum is freed by a single drain op.
  - PSUM: h0,h1,m,l each [128,1024] f32 (2 banks) single-buffered =
    all 8 banks; the skew gives each psum tile a pipeline step to drain.
  - Last tile stores per-pair (strided z/m/l stripes) to shrink the
    final DMA tail.
"""

import sys

sys.path.insert(0, "/opt/trn_rl_repo")

import numpy as np
import ml_dtypes

from contextlib import ExitStack

from concourse import bacc, bass, mybir, tile
from concourse.alu_op_type import AluOpType
from concourse.bass_utils import run_bass_kernel_spmd

N_CORES = 8
B = 524288
D_IN = 128
D_H = 256
D_OUT = 128
ROWS_PER_CORE = B // N_CORES  # 65536
R_DMA = 4096  # rows per DMA super-tile
R_PAIR = 1024  # rows per compute pair-group (epilogue granularity)
R_SUB = 512  # rows per matmul slice (PSUM bank limit)
N_T = ROWS_PER_CORE // R_DMA  # 16
N_U = R_DMA // R_PAIR  # 4 pair-groups per DMA tile

F32 = mybir.dt.float32
BF16 = mybir.dt.bfloat16
NP_BF16 = ml_dtypes.bfloat16

AF = mybir.ActivationFunctionType


def build_bass(rows_per_core=ROWS_PER_CORE):
    nc = bacc.Bacc("TRN2", target_bir_lowering=False, debug=False)
    n_t = rows_per_core // R_DMA

    # per tile: 2048 bf16-slots carrying 4096 fp8e3 x bytes, then 4096 bf16 eps
    IN_T = R_DMA // 2 + R_DMA  # 6144 bf16 units per tile
    in_ext = nc.declare_dram_parameter("inT", [128, n_t * IN_T], BF16, isOutput=False)
    wb_ext = nc.declare_dram_parameter("wb", [128, 768], BF16, isOutput=False)
    bias_ext = nc.declare_dram_parameter("biases", [128, 5], F32, isOutput=False)
    out_ext = nc.declare_dram_parameter(
        "outT", [128, n_t * 3 * R_DMA], BF16, isOutput=True
    )

    with tile.TileContext(nc) as tc, ExitStack() as ctx:
        const = ctx.enter_context(tc.tile_pool(name="const", bufs=1))
        xin = ctx.enter_context(tc.tile_pool(name="xin", bufs=4))
        hpool = ctx.enter_context(tc.tile_pool(name="hT", bufs=5))
        spool = ctx.enter_context(tc.tile_pool(name="small", bufs=4))
        stg = ctx.enter_context(tc.tile_pool(name="stg", bufs=3))
        psH = ctx.enter_context(tc.tile_pool(name="psH", bufs=1, space="PSUM"))
        psO = ctx.enter_context(tc.tile_pool(name="psO", bufs=1, space="PSUM"))

        # --- constants / weights (loaded once, two DMAs) ---
        wb_sb = const.tile([128, 768], BF16)
        nc.sync.dma_start(wb_sb[:], wb_ext[:])
        bias_sb = const.tile([128, 5], F32)
        nc.sync.dma_start(bias_sb[:], bias_ext[:])

        We0 = wb_sb[:, 0:128]
        We1 = wb_sb[:, 128:256]
        Wm0 = wb_sb[:, 256:384]
        Wm1 = wb_sb[:, 384:512]
        Wl0 = wb_sb[:, 512:640]
        Wl1 = wb_sb[:, 640:768]
        be0 = bias_sb[:, 0:1]
        be1 = bias_sb[:, 1:2]
        bm = bias_sb[:, 2:3]
        bl = bias_sb[:, 3:4]
        blh = bias_sb[:, 4:5]

        def mm4(ps, W0, W1, hs0, hs1):
            for v in range(2):
                hs = slice(v * R_SUB, (v + 1) * R_SUB)
                nc.tensor.matmul(
                    ps[:, hs], W0, hs0[:, hs],
                    start=True, stop=False, skip_group_check=True,
                )
            for v in range(2):
                hs = slice(v * R_SUB, (v + 1) * R_SUB)
                nc.tensor.matmul(
                    ps[:, hs], W1, hs1[:, hs],
                    start=False, stop=True, skip_group_check=True,
                )

        def emit_step(k, j, i, iz):
            """One software-pipeline step: z of pair iz (= k-SKEW-2), se of
            pair i (= k-SKEW-1), L2 of pair j (= k-SKEW), L1 of pair k —
            interleaved so each engine's queue starts with long-ready work:
              PE : [L1h0(k), L2m(j), L1h1(k), L2l(j)]
              ACT: [m_out(j), relu0(k), std(j)]
              DVE: [z(iz), l_out(j), relu1(k)]
              GPS: [se(i), store]
            """
            # --- 0. z(iz) = mean + se on GPS ---
            if iz is not None and iz not in pend_z:
                iz = None
            if iz is not None:
                tz, uz = divmod(iz, N_U)
                se_sb_z, o_st_z = pend_z.pop(iz)
                z0, m0 = uz * R_PAIR, R_DMA + uz * R_PAIR
                nc.gpsimd.tensor_tensor(
                    o_st_z[:, z0 : z0 + R_PAIR],
                    o_st_z[:, m0 : m0 + R_PAIR],
                    se_sb_z[:],
                    AluOpType.add,
                )
                if tz == n_t - 1:
                    # last tile: store per pair (strided z/m/l stripes) so the
                    # final DMA tail is ~1/4 the size
                    o3 = out_ext[:, tz * 3 * R_DMA : (tz + 1) * 3 * R_DMA]
                    o3 = o3.rearrange("p (s r) -> p s r", s=3)
                    s_st = o_st_z.rearrange("p (s r) -> p s r", s=3)
                    nc.gpsimd.dma_start(
                        o3[:, :, uz * R_PAIR : (uz + 1) * R_PAIR],
                        s_st[:, :, uz * R_PAIR : (uz + 1) * R_PAIR],
                    )
                    if uz == N_U - 1:
                        tiles.pop(tz)
                elif uz == N_U - 1:
                    store_tile(tz)

            # --- 1. se(i) = std * eps on DVE (bf16 SBUF TT, 2x mode) ---
            if i is not None and i not in pend_tail:
                i = None
            if i is not None:
                ti, ui = divmod(i, N_U)
                std_sb, in_sb_i, o_st_i = pend_tail.pop(i)
                eps0 = R_DMA // 2 + ui * R_PAIR
                eps_ap = in_sb_i[:, eps0 : eps0 + R_PAIR]
                se_sb = spool.tile([128, R_PAIR], BF16, tag="se")
                nc.vector.tensor_tensor(se_sb[:], std_sb[:], eps_ap, AluOpType.mult)
                pend_z[i] = (se_sb, o_st_i)

            if k is not None:
                t, u = divmod(k, N_U)
                if u == 0:
                    load_tile(t)
                in_sb_k = tiles[t][0]
                x_fp8 = in_sb_k[:, 0 : R_DMA // 2].bitcast(mybir.dt.float8e3)
                xs = [
                    x_fp8[:, u * R_PAIR + v * R_SUB : u * R_PAIR + (v + 1) * R_SUB]
                    for v in range(2)
                ]
                # --- 2. L1 h0 matmuls ---
                h_ps0 = psH.tile([128, R_PAIR], F32, tag="h0")
                for v in range(2):
                    hs = slice(v * R_SUB, (v + 1) * R_SUB)
                    nc.tensor.matmul(
                        h_ps0[:, hs], We0, xs[v],
                        start=True, stop=True, skip_group_check=True,
                    )

            if j is not None:
                tj, uj = divmod(j, N_U)
                h_sb0, h_sb1 = pend.pop(j)
                in_sb_j, o_st_j = tiles[tj]
                sl_m = slice(R_DMA + uj * R_PAIR, R_DMA + (uj + 1) * R_PAIR)
                sl_l = slice(2 * R_DMA + uj * R_PAIR, 2 * R_DMA + (uj + 1) * R_PAIR)
                # --- 3. L2 mean matmuls, 4. mean drain ---
                m_ps = psO.tile([128, R_PAIR], F32, tag="m")
                mm4(m_ps, Wm0, Wm1, h_sb0, h_sb1)
                nc.scalar.activation(o_st_j[:, sl_m], m_ps[:], AF.Identity, bias=bm)

            if k is not None:
                # --- 5. relu0(k) ---
                h_sb0k = hpool.tile([128, R_PAIR], BF16, tag="h0")
                nc.scalar.activation(h_sb0k[:], h_ps0[:], AF.Relu, bias=be0)
                # --- 6. L1 h1 matmuls ---
                h_ps1 = psH.tile([128, R_PAIR], F32, tag="h1")
                for v in range(2):
                    hs = slice(v * R_SUB, (v + 1) * R_SUB)
                    nc.tensor.matmul(
                        h_ps1[:, hs], We1, xs[v],
                        start=True, stop=True, skip_group_check=True,
                    )

            if j is not None:
                # --- 7. L2 logvar matmuls, 8. logvar drain (DVE does most
                # columns; a small slice goes to ACT to balance busy time) ---
                l_ps = psO.tile([128, R_PAIR], F32, tag="l")
                mm4(l_ps, Wl0, Wl1, h_sb0, h_sb1)
                nc.vector.tensor_scalar(
                    o_st_j[:, sl_l], l_ps[:], bl, None, AluOpType.add
                )

            if k is not None:
                # --- 9. relu1(k) ---
                h_sb1k = hpool.tile([128, R_PAIR], BF16, tag="h1")
                nc.vector.tensor_scalar(
                    h_sb1k[:], h_ps1[:], be1, 0.0, AluOpType.add, AluOpType.max
                )
                pend[k] = (h_sb0k, h_sb1k)

            if j is not None:
                # --- 10. std(j) = exp(0.5*(lv+bl)) from the SBUF logvar copy;
                # l_ps is already free after one read ---
                std_sb = spool.tile([128, R_PAIR], BF16, tag="std")
                nc.scalar.activation(
                    std_sb[:], o_st_j[:, sl_l], AF.Exp, bias=0.0, scale=0.5
                )
                pend_tail[j] = (std_sb, in_sb_j, o_st_j)

        # flat pair-index loop with a 2-pair software-pipeline skew: L2 of
        # pair k-2 is emitted after L1 of pair k, so the single-buffered
        # PSUM tiles get a full pipeline step to drain before reuse
        n_pairs = n_t * N_U
        SKEW = 3
        tiles = {}  # tile index -> (in_sb, o_st)
        pend = {}  # pair index -> (h0, h1)
        pend_tail = {}  # pair index -> (std_sb, in_sb, o_st)
        pend_z = {}  # pair index -> (se_sb, o_st)

        def load_tile(t):
            in_sb = xin.tile([128, IN_T], BF16, tag="in")
            nc.sync.dma_start(in_sb[:], in_ext[:, t * IN_T : (t + 1) * IN_T])
            o_st = stg.tile([128, 3 * R_DMA], BF16, tag="o")
            tiles[t] = (in_sb, o_st)

        def store_tile(t):
            _, o_st = tiles.pop(t)
            nc.gpsimd.dma_start(
                out_ext[:, t * 3 * R_DMA : (t + 1) * 3 * R_DMA], o_st[:]
            )

        for k in range(n_pairs + SKEW + 2):
            j = k - SKEW
            i = k - SKEW - 1
            iz = k - SKEW - 2
            emit_step(
                k if k < n_pairs else None,
                j if 0 <= j < n_pairs else None,
                i if 0 <= i < n_pairs else None,
                iz if iz >= 0 else None,
            )

    nc.finalize()
    return nc


_NC_CACHE = None


def _get_nc():
    global _NC_CACHE
    if _NC_CACHE is None:
        _NC_CACHE = build_bass()
    return _NC_CACHE


def _run(inputs, trace=False, **kw):
    nc = _get_nc()
    f32 = np.float32
    x = np.asarray(inputs["x"], dtype=f32)
    eps = np.asarray(inputs["eps"], dtype=f32)

    # x is shipped as fp8-e3m4 scaled by 2; compensate with We/2 (exact in bf16)
    We = (np.asarray(inputs["W_emb"], f32) * 0.5).astype(NP_BF16)  # [128,256]
    Wm = np.asarray(inputs["W_mean"], f32).astype(NP_BF16)  # [256,128]
    Wl = np.asarray(inputs["W_logvar"], f32).astype(NP_BF16)
    Wm_r = Wm.reshape(2, 128, 128).transpose(1, 0, 2).reshape(128, 256)
    Wl_r = Wl.reshape(2, 128, 128).transpose(1, 0, 2).reshape(128, 256)
    wb = np.ascontiguousarray(np.concatenate([We, Wm_r, Wl_r], axis=1))

    be = np.asarray(inputs["b_emb"], f32)
    bm = np.asarray(inputs["b_mean"], f32)
    bl = np.asarray(inputs["b_logvar"], f32)
    biases = np.ascontiguousarray(
        np.stack([be[0:128], be[128:256], bm, bl, 0.5 * bl], axis=1)
    )  # [128, 5]

    NP_E3M4 = ml_dtypes.float8_e3m4
    IN_T = R_DMA // 2 + R_DMA  # bf16 units per tile (x-fp8 bytes + eps)
    weights = {"wb": wb, "biases": biases}
    in_maps = []
    for c in range(N_CORES):
        sl = slice(c * ROWS_PER_CORE, (c + 1) * ROWS_PER_CORE)
        xT8 = np.ascontiguousarray((x[sl].T * 2.0).astype(NP_E3M4))  # [128, 65536]
        epsT = np.ascontiguousarray(eps[sl].T.astype(NP_BF16))
        xb = xT8.view(np.uint8).reshape(128, N_T, R_DMA)
        eb = epsT.view(np.uint8).reshape(128, N_T, 2 * R_DMA)
        inT = (
            np.concatenate([xb, eb], axis=2)
            .reshape(128, N_T * 3 * R_DMA)
            .copy()
            .view(NP_BF16)
        )
        in_maps.append({"inT": inT, **weights})
    res = run_bass_kernel_spmd(nc, in_maps, list(range(N_CORES)), trace=trace, **kw)

    outs = [np.empty((B, D_OUT), dtype=f32) for _ in range(3)]
    for c in range(N_CORES):
        sl = slice(c * ROWS_PER_CORE, (c + 1) * ROWS_PER_CORE)
        o = np.asarray(res.results[c]["outT"]).reshape(128, N_T, 3, R_DMA)
        for i in range(3):
            outs[i][sl] = o[:, :, i, :].reshape(128, ROWS_PER_CORE).T.astype(f32)
    return tuple(outs), res


def kernel(**inputs):
    out, _ = _run(inputs, trace=False)
    return out


if __name__ == "__main__":
    rng = np.random.default_rng(0)
    demo = {
        "x": rng.standard_normal((B, D_IN), dtype=np.float32),
        "eps": rng.standard_normal((B, D_OUT), dtype=np.float32),
        "W_emb": rng.standard_normal((D_IN, D_H), dtype=np.float32) * 0.088,
        "b_emb": rng.standard_normal((D_H,), dtype=np.float32) * 0.05,
        "W_mean": rng.standard_normal((D_H, D_OUT), dtype=np.float32) * 0.06,
        "b_mean": rng.standard_normal((D_OUT,), dtype=np.float32) * 0.03,
        "W_logvar": rng.standard_normal((D_H, D_OUT), dtype=np.float32) * 0.06,
        "b_logvar": rng.standard_normal((D_OUT,), dtype=np.float32) * 0.03,
    }
    z, m, l = kernel(**demo)
    print("shapes", z.shape, m.shape, l.shape, z.dtype)
